# revision 15
# baseline (speedup 1.0000x reference)
import os
import numpy as np

# GCN regressor on 8 trn2 NeuronCores.
# Formulation: GCNConv commutes with the dense W-matmul, so aggregate first:
#   agg[v] = dinv[v] * (sum_{e: dst=v} xt[src_e] + xt[v]),  xt = dinv * x
#   h = agg @ W   (conv bias cancels inside BatchNorm)
# Edges are dst-sharded across cores (12500 dst nodes/core), sorted into
# 512-dst windows, padded to K chunks of 128 edges. Each chunk: indirect-DMA
# gather of 128 table rows + one-hot selection matmul accumulating into PSUM.
# BN stats via AllReduce; features AllGathered between layers; mean-pool and
# the MLP head also run on-device via selection matmuls.

N = 100000
E = 1600000
G = 512
F_IN = 30
H = 128
EPS = 1e-5
NC = 8
SHARD = 12500          # real dst nodes per core
WIN = 512              # dst nodes per window
NWIN = 25              # windows per core (25*512 = 12800 >= 12500)
LSH = NWIN * WIN       # padded local shard rows
TROWS = NC * LSH       # padded table rows
F0 = 32                # layer-0 feature dim padded 30 -> 32

_CACHE = {}


def _preprocess(x, edge_index, batch):
    src = edge_index[0].astype(np.int64)
    dst = edge_index[1].astype(np.int64)
    deg = np.bincount(dst, minlength=N).astype(np.float32) + 1.0
    dinv = (1.0 / np.sqrt(deg)).astype(np.float32)

    core = dst // SHARD
    local = dst - core * SHARD
    win = local // WIN
    ldst = local - win * WIN                      # 0..511 within window
    csrc = src // SHARD
    trow = (csrc * LSH + (src - csrc * SHARD)).astype(np.int32)

    key = (core * NWIN + win).astype(np.int64)
    order = np.argsort(key, kind="stable")
    cnt = np.bincount(key, minlength=NC * NWIN)
    K = int(np.ceil(cnt.max() / 128))
    C = NWIN * K

    start = np.zeros(NC * NWIN, np.int64)
    start[1:] = np.cumsum(cnt)[:-1]
    rank = np.empty(E, np.int64)
    rank[order] = np.arange(E) - start[key[order]]

    srcidx = np.zeros((NC, 128, C), np.int32)
    ld = np.full((NC, 128, C), -1.0, np.float32)
    part = rank % 128
    col = (win * K + rank // 128).astype(np.int64)
    srcidx[core, part, col] = trow
    ld[core, part, col] = ldst.astype(np.float32)

    # layer-0 gather table (dinv-scaled, remapped rows, padded)
    x0t = np.zeros((TROWS, F0), np.float32)
    n = np.arange(N)
    rows = (n // SHARD) * LSH + (n % SHARD)
    x0t[rows, :F_IN] = x * dinv[:, None]

    xown0 = np.stack([x0t[c * LSH:(c + 1) * LSH] for c in range(NC)])

    dv_full = np.zeros((NC, LSH), np.float32)
    bl_full = np.full((NC, LSH), -1.0, np.float32)
    for c in range(NC):
        dv_full[c, :SHARD] = dinv[c * SHARD:(c + 1) * SHARD]
        bl_full[c, :SHARD] = batch[c * SHARD:(c + 1) * SHARD].astype(np.float32)
    dinvb = np.broadcast_to(dv_full[:, None, :], (NC, 128, LSH)).copy()
    batchcol = np.ascontiguousarray(
        bl_full.reshape(NC, LSH // 128, 128).transpose(0, 2, 1))

    return dict(K=K, C=C, srcidx=srcidx, ld=ld, x0t=x0t, xown0=xown0,
                dinvb=dinvb, batchcol=batchcol)


def _build(K):
    import concourse.bass as bass
    import concourse.tile as tile
    from concourse import bacc, mybir
    f32 = mybir.dt.float32
    C = NWIN * K

    nc = bacc.Bacc("TRN2", target_bir_lowering=False, debug=False,
                   enable_asserts=False, num_devices=NC)
    din = {}
    def dt(name, shape, dty=f32, kind="ExternalInput"):
        din[name] = nc.dram_tensor(name, shape, dty, kind=kind).ap()
        return din[name]

    x0t = dt("x0t", [TROWS, F0])
    xown0 = dt("xown0", [LSH, F0])
    srcidx = dt("srcidx", [128, C], mybir.dt.int32)
    ldv = dt("ldv", [128, C])
    dinvb = dt("dinvb", [128, LSH])
    batchcol = dt("batchcol", [128, LSH // 128])
    iota = dt("iota", [128, WIN])
    ident = dt("ident", [128, 128])
    onescol = dt("onescol", [128, 1])
    onesrow = dt("onesrow", [1, 64])
    Ws = [dt("W0", [F0, H]), dt("W1", [H, H]), dt("W2", [H, H])]
    gs = [dt(f"g{l}", [H, 1]) for l in range(3)]
    bts = [dt(f"bt{l}", [H, 1]) for l in range(3)]
    HW1 = dt("HW1", [H, 64])
    Hb1 = dt("Hb1", [64, 1])
    HW2 = dt("HW2", [64, 1])
    Hb2 = dt("Hb2", [1, 1])
    out = dt("out", [1, G], kind="ExternalOutput")

    AX = mybir.AxisListType.X
    OP = mybir.AluOpType
    ACTF = mybir.ActivationFunctionType

    with tile.TileContext(nc) as tc:
        with tc.tile_pool(name="const", bufs=1) as cp, \
             tc.tile_pool(name="big", bufs=1) as bigp, \
             tc.tile_pool(name="dram", bufs=1, space="DRAM") as dp, \
             tc.tile_pool(name="gat", bufs=8) as gp, \
             tc.tile_pool(name="selp", bufs=8) as sp, \
             tc.tile_pool(name="work", bufs=2) as wp, \
             tc.tile_pool(name="stat", bufs=1) as stp, \
             tc.tile_pool(name="ps2", bufs=2, space="PSUM") as ps2, \
             tc.tile_pool(name="ps1", bufs=1, space="PSUM") as ps1:

            def csb(ap, shape, dty=f32, name="c"):
                t = cp.tile(shape, dty, name=name)
                nc.sync.dma_start(out=t[:], in_=ap[:])
                return t

            srci = csb(srcidx, [128, C], mybir.dt.int32, "srci")
            bcc = csb(batchcol, [128, LSH // 128], name="bcc")
            lds = csb(ldv, [128, C], name="lds")
            iot = csb(iota, [128, WIN], name="iot")
            idn = csb(ident, [128, 128], name="idn")
            oc = csb(onescol, [128, 1], name="oc")
            orow = csb(onesrow, [1, 64], name="orow")
            Wsb = [csb(Ws[l], [F0 if l == 0 else H, H], name=f"w{l}sb")
                   for l in range(3)]
            gsb = [csb(gs[l], [H, 1], name=f"g{l}sb") for l in range(3)]
            btsb = [csb(bts[l], [H, 1], name=f"bt{l}sb") for l in range(3)]
            hw1 = csb(HW1, [H, 64], name="hw1")
            hb1 = csb(Hb1, [64, 1], name="hb1")
            hw2 = csb(HW2, [64, 1], name="hw2")
            hb2 = csb(Hb2, [1, 1], name="hb2")

            hT = bigp.tile([128, LSH], f32, name="hT")

            shards = [None,
                      dp.tile([LSH, H], f32, name="t1s"),
                      dp.tile([LSH, H], f32, name="t2s")]
            fulls = [None,
                     dp.tile([TROWS, H], f32, addr_space="Shared", name="t1f"),
                     dp.tile([TROWS, H], f32, addr_space="Shared", name="t2f")]
            stats_in = dp.tile([128, 2], f32, name="stats_in")
            stats_out = [dp.tile([128, 2], f32, addr_space="Shared",
                                 name=f"stats_out{l}") for l in range(3)]
            pool_in = dp.tile([129, G], f32, name="pool_in")
            pool_out = dp.tile([129, G], f32, addr_space="Shared",
                               name="pool_out")

            poolps = ps1.tile([128, G], f32, name="poolps")
            cntps = ps1.tile([1, G], f32, name="cntps")

            rg = [list(range(NC))]

            for l in range(3):
                F = F0 if l == 0 else H
                table = x0t if l == 0 else fulls[l][:]
                own = xown0 if l == 0 else shards[l][:]

                sums = stp.tile([128, 1], f32, name=f"sums{l}")
                sumsq = stp.tile([128, 1], f32, name=f"sumsq{l}")

                for w in range(NWIN):
                    yps = ps2.tile([128, WIN], f32, tag="yps", name="yps")
                    for j in range(K):
                        col = w * K + j
                        g = gp.tile([128, H], f32, tag="g", name="g")
                        nc.gpsimd.indirect_dma_start(
                            out=g[:, :F], out_offset=None, in_=table,
                            in_offset=bass.IndirectOffsetOnAxis(
                                ap=srci[:, col:col + 1], axis=0))
                        sel = sp.tile([128, WIN], f32, tag="sel", name="sel")
                        nc.vector.tensor_tensor(
                            out=sel[:],
                            in0=lds[:, col:col + 1].to_broadcast([128, WIN]),
                            in1=iot[:], op=OP.is_equal)
                        nc.tensor.matmul(yps[:F, :], lhsT=g[:, :F], rhs=sel[:],
                                         start=(j == 0), stop=(j == K - 1))
                    xps = ps2.tile([128, WIN], f32, tag="xps", name="xps",
                                   bufs=1)
                    for s in range(4):
                        xw = wp.tile([128, F0 if l == 0 else H], f32,
                                     tag="xw", name="xw")
                        nc.sync.dma_start(
                            out=xw[:, :F],
                            in_=own[w * WIN + s * 128: w * WIN + (s + 1) * 128, :F])
                        nc.tensor.transpose(
                            out=xps[:F, s * 128:(s + 1) * 128],
                            in_=xw[:, :F], identity=idn[:])
                    xsb = wp.tile([128, WIN], f32, tag="xsb", name="xsb")
                    nc.scalar.copy(xsb[:F, :], xps[:F, :])
                    y = wp.tile([128, WIN], f32, tag="y", name="y")
                    nc.vector.tensor_tensor(out=y[:F, :], in0=yps[:F, :],
                                            in1=xsb[:F, :], op=OP.add)
                    dv = wp.tile([128, WIN], f32, tag="dv", name="dv")
                    nc.sync.dma_start(out=dv[:F, :],
                                      in_=dinvb[:F, w * WIN:(w + 1) * WIN])
                    nc.vector.tensor_tensor(out=y[:F, :], in0=y[:F, :],
                                            in1=dv[:F, :], op=OP.mult)
                    hps = ps2.tile([128, WIN], f32, tag="hps", name="hps",
                                   bufs=1)
                    nc.tensor.matmul(hps[:], lhsT=Wsb[l][:F, :], rhs=y[:F, :],
                                     start=True, stop=True)
                    nc.scalar.copy(hT[:, w * WIN:(w + 1) * WIN], hps[:])
                    hsb = hT[:, w * WIN:(w + 1) * WIN]
                    sq = wp.tile([128, WIN], f32, tag="sq", name="sq")
                    nc.scalar.activation(sq[:], hsb, ACTF.Square)
                    rs = wp.tile([128, 1], f32, tag="rs", name="rs")
                    rq = wp.tile([128, 1], f32, tag="rq", name="rq")
                    nc.vector.reduce_sum(rs[:], hsb, axis=AX)
                    nc.vector.reduce_sum(rq[:], sq[:], axis=AX)
                    if w == 0:
                        nc.vector.tensor_copy(sums[:], rs[:])
                        nc.vector.tensor_copy(sumsq[:], rq[:])
                    else:
                        nc.vector.tensor_tensor(out=sums[:], in0=sums[:],
                                                in1=rs[:], op=OP.add)
                        nc.vector.tensor_tensor(out=sumsq[:], in0=sumsq[:],
                                                in1=rq[:], op=OP.add)

                # BN stats allreduce
                stp2 = wp.tile([128, 2], f32, tag="stp2", name="stp2")
                nc.vector.tensor_copy(stp2[:, 0:1], sums[:])
                nc.vector.tensor_copy(stp2[:, 1:2], sumsq[:])
                nc.gpsimd.dma_start(out=stats_in[:], in_=stp2[:])
                nc.gpsimd.collective_compute(
                    "AllReduce", OP.add, replica_groups=rg,
                    ins=[stats_in.opt()], outs=[stats_out[l].opt()])
                st = wp.tile([128, 2], f32, tag="st", name="st")
                nc.gpsimd.dma_start(out=st[:], in_=stats_out[l][:])
                mu = stp.tile([128, 1], f32, name=f"mu{l}")
                ex2 = stp.tile([128, 1], f32, name=f"ex2{l}")
                nc.vector.tensor_scalar_mul(mu[:], st[:, 0:1], 1.0 / N)
                nc.vector.tensor_scalar_mul(ex2[:], st[:, 1:2], 1.0 / N)
                var = stp.tile([128, 1], f32, name=f"var{l}")
                nc.vector.tensor_tensor(out=var[:], in0=mu[:], in1=mu[:],
                                        op=OP.mult)
                nc.vector.tensor_tensor(out=var[:], in0=ex2[:], in1=var[:],
                                        op=OP.subtract)
                nc.vector.tensor_scalar_add(var[:], var[:], EPS)
                rv = stp.tile([128, 1], f32, name=f"rv{l}")
                nc.vector.reciprocal(rv[:], var[:])
                rstd = stp.tile([128, 1], f32, name=f"rstd{l}")
                nc.scalar.activation(rstd[:], rv[:], ACTF.Sqrt)
                rsg = stp.tile([128, 1], f32, name=f"rsg{l}")
                nc.vector.tensor_tensor(out=rsg[:], in0=rstd[:], in1=gsb[l][:],
                                        op=OP.mult)
                bia = stp.tile([128, 1], f32, name=f"bia{l}")
                nc.vector.tensor_tensor(out=bia[:], in0=mu[:], in1=rsg[:],
                                        op=OP.mult)
                nc.vector.tensor_tensor(out=bia[:], in0=btsb[l][:], in1=bia[:],
                                        op=OP.subtract)

                # BN apply + relu (+ dinv for l<2), transpose, write/pool
                for w in range(NWIN):
                    xa = wp.tile([128, WIN], f32, tag="xa", name="xa")
                    nc.scalar.activation(xa[:], hT[:, w * WIN:(w + 1) * WIN],
                                         ACTF.Relu, bias=bia[:], scale=rsg[:])
                    if l < 2:
                        dv2 = wp.tile([128, WIN], f32, tag="dv2", name="dv2")
                        nc.sync.dma_start(out=dv2[:],
                                          in_=dinvb[:, w * WIN:(w + 1) * WIN])
                        nc.vector.tensor_tensor(out=xa[:], in0=xa[:],
                                                in1=dv2[:], op=OP.mult)
                    for s in range(4):
                        tps = ps2.tile([128, 128], f32, tag="tps", name="tps",
                                       bufs=1)
                        nc.tensor.transpose(
                            out=tps[:], in_=xa[:, s * 128:(s + 1) * 128],
                            identity=idn[:])
                        xn = wp.tile([128, 128], f32, tag="xn", name="xn")
                        nc.scalar.copy(xn[:], tps[:])
                        if l < 2:
                            nc.sync.dma_start(
                                out=shards[l + 1][w * WIN + s * 128:
                                                  w * WIN + (s + 1) * 128, :],
                                in_=xn[:])
                        else:
                            sw = w * 4 + s
                            psel = sp.tile([128, G], f32, tag="psel",
                                           name="psel")
                            nc.vector.tensor_tensor(
                                out=psel[:],
                                in0=bcc[:, sw:sw + 1]
                                    .to_broadcast([128, G]),
                                in1=iot[:, :G], op=OP.is_equal)
                            first = (w == 0 and s == 0)
                            last = (w == NWIN - 1 and s == 3)
                            nc.tensor.matmul(poolps[:], lhsT=xn[:],
                                             rhs=psel[:], start=first,
                                             stop=last)
                            nc.tensor.matmul(cntps[:], lhsT=oc[:], rhs=psel[:],
                                             start=first, stop=last)
                if l < 2:
                    nc.gpsimd.collective_compute(
                        "AllGather", OP.bypass, replica_groups=rg,
                        ins=[shards[l + 1].opt()], outs=[fulls[l + 1].opt()])

            # pooled allreduce + head
            pl = wp.tile([128, G], f32, tag="pl", name="pl")
            cn = wp.tile([1, G], f32, tag="cn", name="cn")
            nc.scalar.copy(pl[:], poolps[:])
            nc.scalar.copy(cn[:], cntps[:])
            nc.gpsimd.dma_start(out=pool_in[0:128, :], in_=pl[:])
            nc.gpsimd.dma_start(out=pool_in[128:129, :], in_=cn[:])
            nc.gpsimd.collective_compute(
                "AllReduce", OP.add, replica_groups=rg,
                ins=[pool_in.opt()], outs=[pool_out.opt()])
            plr = wp.tile([128, G], f32, tag="plr", name="plr")
            cnr = wp.tile([1, G], f32, tag="cnr", name="cnr")
            nc.sync.dma_start(out=plr[:], in_=pool_out[0:128, :])
            nc.sync.dma_start(out=cnr[:], in_=pool_out[128:129, :])
            nc.vector.tensor_scalar_max(cnr[:], cnr[:], 1.0)
            inv = wp.tile([1, G], f32, tag="inv", name="inv")
            nc.vector.reciprocal(inv[:], cnr[:])
            invb = ps2.tile([64, G], f32, tag="yps", name="invb")
            nc.tensor.matmul(invb[:], lhsT=orow[:], rhs=inv[:],
                             start=True, stop=True)
            h1ps = ps2.tile([64, G], f32, tag="xps", name="h1ps", bufs=1)
            nc.tensor.matmul(h1ps[:], lhsT=hw1[:], rhs=plr[:],
                             start=True, stop=True)
            invs = wp.tile([64, G], f32, tag="invs", name="invs")
            nc.scalar.copy(invs[:], invb[:])
            h1a = wp.tile([64, G], f32, tag="h1a", name="h1a")
            nc.vector.tensor_tensor(out=h1a[:], in0=h1ps[:], in1=invs[:],
                                    op=OP.mult)
            nc.scalar.activation(h1a[:], h1a[:], ACTF.Relu, bias=hb1[:])
            ops = ps2.tile([1, G], f32, tag="hps", name="ops", bufs=1)
            nc.tensor.matmul(ops[:], lhsT=hw2[:], rhs=h1a[:],
                             start=True, stop=True)
            osb = wp.tile([1, G], f32, tag="osb", name="osb")
            nc.vector.tensor_scalar(out=osb[:], in0=ops[:], scalar1=hb2[:],
                                    scalar2=None, op0=OP.add)
            nc.sync.dma_start(out=out[:], in_=osb[:])

    nc.compile()
    return nc


def kernel(x, edge_index, batch, W0, b0, g0, bt0, W1, b1, g1, bt1,
           W2, b2, g2, bt2, HW1, Hb1, HW2, Hb2):
    x = np.asarray(x, np.float32)
    edge_index = np.asarray(edge_index)
    batch = np.asarray(batch)
    pp = _preprocess(x, edge_index, batch)
    K = pp["K"]

    if _CACHE.get("K") != K:
        _CACHE["nc"] = _build(K)
        _CACHE["K"] = K
    nc = _CACHE["nc"]

    W0p = np.zeros((F0, H), np.float32)
    W0p[:F_IN] = np.asarray(W0, np.float32)
    com = {
        "x0t": pp["x0t"],
        "iota": np.broadcast_to(np.arange(WIN, dtype=np.float32),
                                (128, WIN)).copy(),
        "ident": np.eye(128, dtype=np.float32),
        "onescol": np.ones((128, 1), np.float32),
        "onesrow": np.ones((1, 64), np.float32),
        "W0": W0p, "W1": np.asarray(W1, np.float32),
        "W2": np.asarray(W2, np.float32),
        "g0": np.asarray(g0, np.float32).reshape(H, 1),
        "g1": np.asarray(g1, np.float32).reshape(H, 1),
        "g2": np.asarray(g2, np.float32).reshape(H, 1),
        "bt0": np.asarray(bt0, np.float32).reshape(H, 1),
        "bt1": np.asarray(bt1, np.float32).reshape(H, 1),
        "bt2": np.asarray(bt2, np.float32).reshape(H, 1),
        "HW1": np.asarray(HW1, np.float32),
        "Hb1": np.asarray(Hb1, np.float32).reshape(64, 1),
        "HW2": np.asarray(HW2, np.float32),
        "Hb2": np.asarray(Hb2, np.float32).reshape(1, 1),
    }
    ins = []
    for c in range(NC):
        m = dict(com)
        m["srcidx"] = pp["srcidx"][c]
        m["ldv"] = pp["ld"][c]
        m["dinvb"] = pp["dinvb"][c]
        m["batchcol"] = pp["batchcol"][c]
        m["xown0"] = pp["xown0"][c]
        ins.append(m)

    from concourse.bass_utils import run_bass_kernel_spmd
    trace = bool(int(os.environ.get("BASSGCN_TRACE", "0")))
    res = run_bass_kernel_spmd(nc, ins, core_ids=list(range(NC)), trace=trace)
    if trace:
        _CACHE["exec_ns"] = res.exec_time_ns
    return res.results[0]["out"].reshape(G).astype(np.float32)
